# revision 1
# baseline (speedup 1.0000x reference)
"""MoE FFN (DeepSeek-style top-2 routing + shared expert) on 8 TRN2 cores.

Sharding: expert-parallel for the 8 routed experts (core e owns expert e,
host gathers/pads its top-2 tokens to a fixed capacity C); the shared
expert is split 2 token-halves x 4 F-quarters (384 F-rows each) so its
weights stay tiny and SBUF-resident. Host does router + dispatch/combine
(the unshard step); device does all FLOPs-heavy matmuls.

Self-contained: hardcodes B=2,S=2048,D=768,E=8,K=2,F=1536.
"""
import ml_dtypes
import numpy as np
from contextlib import ExitStack

import concourse.bacc as bacc
import concourse.mybir as mybir
import concourse.tile as tile
from concourse.bass import ts
from concourse.bass_utils import run_bass_kernel_spmd

B, S, D = 2, 2048, 768
E, TOPK, F = 8, 2, 1536
T = B * S
NCORES = 8
KD = D // 128           # 6 contraction chunks over D
MF = F // 128           # 12 f-tiles for routed experts
MD = D // 128           # 6 output d-tiles
FS = 384                # shared-expert F-slice per core (4 slices x 2 token halves)
MFS = FS // 128         # 3 f-tiles for shared slice
TH = T // 2             # shared-expert token half
NT = 512                # moving-operand (token) tile

F32 = mybir.dt.float32
COMPUTE_DT = mybir.dt.bfloat16   # matmul operand dtype (bf16: FWL + half DMA)
NP_COMPUTE = np.float32 if COMPUTE_DT == mybir.dt.float32r else ml_dtypes.bfloat16

_cache: dict = {}


def _chunks(total, step=NT):
    out, o = [], 0
    while o < total:
        n = min(step, total - o)
        out.append((o, n))
        o += n
    return out


def _build(C):
    """One SPMD program: routed expert over C tokens + shared slice over TH."""
    nc = bacc.Bacc("TRN2", debug=False)
    xeT = nc.dram_tensor("xeT", [D, C], COMPUTE_DT, kind="ExternalInput")
    wgT = nc.dram_tensor("wgT", [D, F], COMPUTE_DT, kind="ExternalInput")
    wuT = nc.dram_tensor("wuT", [D, F], COMPUTE_DT, kind="ExternalInput")
    wdT = nc.dram_tensor("wdT", [F, D], COMPUTE_DT, kind="ExternalInput")
    xsT = nc.dram_tensor("xsT", [D, TH], COMPUTE_DT, kind="ExternalInput")
    sgT = nc.dram_tensor("sgT", [D, FS], COMPUTE_DT, kind="ExternalInput")
    suT = nc.dram_tensor("suT", [D, FS], COMPUTE_DT, kind="ExternalInput")
    sdT = nc.dram_tensor("sdT", [FS, D], COMPUTE_DT, kind="ExternalInput")
    yeT = nc.dram_tensor("yeT", [D, C], F32, kind="ExternalOutput")
    zT = nc.dram_tensor("zT", [D, TH], F32, kind="ExternalOutput")

    with tile.TileContext(nc) as tc, ExitStack() as ctx:
        wpool = ctx.enter_context(tc.tile_pool(name="w", bufs=1))
        xpool = ctx.enter_context(tc.tile_pool(name="x", bufs=3))
        hpool = ctx.enter_context(tc.tile_pool(name="h", bufs=2))
        spool = ctx.enter_context(tc.tile_pool(name="s", bufs=3))
        opool = ctx.enter_context(tc.tile_pool(name="o", bufs=6))
        pgp = ctx.enter_context(tc.tile_pool(name="pg", bufs=2, space="PSUM"))
        pup = ctx.enter_context(tc.tile_pool(name="pu", bufs=2, space="PSUM"))
        pyp = ctx.enter_context(tc.tile_pool(name="py", bufs=3, space="PSUM"))

        def load_rows(src, width, n, tag):
            tiles = []
            for k in range(n):
                t = wpool.tile([128, width], COMPUTE_DT, tag=f"{tag}{k}")
                nc.sync.dma_start(t[:], src[ts(k, 128), :])
                tiles.append(t)
            return tiles

        # iteration list: shared-expert tiles FIRST (their weights are 8x
        # smaller, so the PE starts ~3.5us in while the big routed-expert
        # weights stream in behind), then the routed tail tile, then full
        # routed tiles (a full tile last shortens the drain).
        s_it = [("S", o, n) for o, n in _chunks(TH)]
        r_it = [("R", o, n) for o, n in _chunks(C)]
        iters = [s_it[0], s_it[1], r_it[0], s_it[2]] + \
                ([r_it[1]] if len(r_it) > 1 else []) + [s_it[3]] + r_it[2:]

        def load_x(ph, o, n):
            x_src = xeT if ph == "R" else xsT
            xt = xpool.tile([128, KD * NT], COMPUTE_DT, tag="xt")
            for k in range(KD):
                # POOL-engine DMA queue: keeps x tiles from FIFO-ing
                # behind the big weight loads on the sync queue.
                nc.gpsimd.dma_start(xt[:, k * NT:k * NT + n],
                                    x_src[ts(k, 128), o:o + n])
            return xt

        # first x tiles before any weight DMA (nothing blocks the PE start)
        xts = [load_x(*iters[0]), load_x(*iters[1])]
        sg_sb = load_rows(sgT, FS, KD, "sg")
        su_sb = load_rows(suT, FS, KD, "su")
        sd_sb = load_rows(sdT, D, MFS, "sd")
        wg_sb = load_rows(wgT, F, KD, "wg")
        wu_sb = load_rows(wuT, F, KD, "wu")
        wd_sb = load_rows(wdT, D, MF, "wd")

        def mm1(ph, o, n, xt):
            """gate/up matmuls + silu/mul -> hT tiles for one iteration."""
            g_w, u_w = (wg_sb, wu_sb) if ph == "R" else (sg_sb, su_sb)
            mf = MF if ph == "R" else MFS
            hT = []
            for m in range(mf):
                g = pgp.tile([128, NT], F32, tag="pg")
                u = pup.tile([128, NT], F32, tag="pu")
                for k in range(KD):
                    nc.tensor.matmul(g[:, :n], g_w[k][:, ts(m, 128)],
                                     xt[:, k * NT:k * NT + n],
                                     start=(k == 0), stop=(k == KD - 1))
                for k in range(KD):
                    nc.tensor.matmul(u[:, :n], u_w[k][:, ts(m, 128)],
                                     xt[:, k * NT:k * NT + n],
                                     start=(k == 0), stop=(k == KD - 1))
                sil = spool.tile([128, NT], F32, tag="sil")
                nc.scalar.activation(sil[:, :n], g[:, :n],
                                     mybir.ActivationFunctionType.Sigmoid)
                gs = spool.tile([128, NT], F32, tag="gs")
                nc.vector.tensor_mul(gs[:, :n], sil[:, :n], g[:, :n])
                h = hpool.tile([128, NT], COMPUTE_DT, tag=f"h{m}")
                nc.vector.tensor_mul(h[:, :n], gs[:, :n], u[:, :n])
                hT.append(h)
            return hT

        def mm2(ph, o, n, hT):
            """down-projection + copy-out for one iteration."""
            out_dst = yeT if ph == "R" else zT
            d_w = wd_sb if ph == "R" else sd_sb
            mf = MF if ph == "R" else MFS
            for m2 in range(MD):
                y = pyp.tile([128, NT], F32, tag="py")
                for k2 in range(mf):
                    nc.tensor.matmul(y[:, :n], d_w[k2][:, ts(m2, 128)],
                                     hT[k2][:, :n],
                                     start=(k2 == 0), stop=(k2 == mf - 1))
                yo = opool.tile([128, NT], F32, tag="yo")
                nc.scalar.copy(yo[:, :n], y[:, :n])
                nc.scalar.dma_start(out_dst[ts(m2, 128), o:o + n], yo[:, :n])

        # software pipeline: emit MM1(i+1) before MM2(i) so the PE chews on
        # the next tile's gate/up while ACT/DVE finish hT(i).
        hprev = None
        for i, it in enumerate(iters):
            if i + 2 < len(iters):
                xts.append(load_x(*iters[i + 2]))
            h = mm1(*it, xts[i])
            if hprev is not None:
                mm2(*iters[i - 1], hprev)
            hprev = h
        mm2(*iters[-1], hprev)
    nc.compile()
    return nc


def _router(xf, w_router, expert_bias):
    """Replicates the reference router. f64 for stable top-k ordering,
    f32 softmax (same formula as jax.nn.softmax) for the weights."""
    logits = xf.astype(np.float64) @ w_router.T.astype(np.float64)
    l32 = (xf @ w_router.T).astype(np.float32)
    m = l32.max(-1, keepdims=True)
    e32 = np.exp(l32 - m)
    scores = e32 / e32.sum(-1, keepdims=True)
    e64 = np.exp(logits - logits.max(-1, keepdims=True))
    sel = e64 / e64.sum(-1, keepdims=True) + expert_bias.astype(np.float64)[None, :]
    top_idx = np.argsort(-sel, axis=-1, kind="stable")[:, :TOPK]
    top_s = np.take_along_axis(scores, top_idx, axis=-1)
    top_s = top_s / (top_s.sum(-1, keepdims=True) + 1e-9)
    return top_idx, top_s


def kernel(x, w_router, expert_bias, Wg, Wu, Wd, sg, su, sd):
    x = np.asarray(x); w_router = np.asarray(w_router)
    expert_bias = np.asarray(expert_bias)
    Wg = np.asarray(Wg); Wu = np.asarray(Wu); Wd = np.asarray(Wd)
    sg = np.asarray(sg); su = np.asarray(su); sd = np.asarray(sd)
    xf = x.reshape(-1, D).astype(np.float32)

    top_idx, top_s = _router(xf, w_router, expert_bias)

    idxs, ws = [], []
    for e in range(E):
        hit = (top_idx == e)
        tok = np.nonzero(hit.any(-1))[0]
        idxs.append(tok)
        ws.append(top_s[tok][hit[tok]])
    cmax = max(len(i) for i in idxs)
    C = max(512, -(-cmax // 128) * 128)

    key = C
    if key not in _cache:
        _cache[key] = _build(C)
    nc = _cache[key]

    cast = lambda a: np.ascontiguousarray(a, dtype=np.float32).astype(NP_COMPUTE)
    in_maps = []
    for e in range(E):
        xeT = np.zeros((D, C), np.float32)
        xeT[:, :len(idxs[e])] = xf[idxs[e]].T
        th, fq = e // 4, e % 4
        in_maps.append({
            "xeT": cast(xeT),
            "wgT": cast(Wg[e].T), "wuT": cast(Wu[e].T), "wdT": cast(Wd[e].T),
            "xsT": cast(xf[th * TH:(th + 1) * TH].T),
            "sgT": cast(sg[fq * FS:(fq + 1) * FS].T),
            "suT": cast(su[fq * FS:(fq + 1) * FS].T),
            "sdT": cast(sd[:, fq * FS:(fq + 1) * FS].T),
        })

    res = run_bass_kernel_spmd(nc, in_maps, core_ids=list(range(NCORES)))

    out = np.zeros((T, D), np.float32)
    for e in range(E):
        ye = res.results[e]["yeT"].T[:len(idxs[e])]
        out[idxs[e]] += ws[e][:, None] * ye
        th = e // 4
        out[th * TH:(th + 1) * TH] += res.results[e]["zT"].T
    return out.reshape(B, S, D).astype(x.dtype)



# revision 13
# speedup vs baseline: 4.5492x; 4.5492x over previous
"""MoE FFN (DeepSeek-style top-2 routing + shared expert) on 8 TRN2 cores.

Sharding: expert-parallel for the 8 routed experts (core e owns expert e,
host gathers/pads its top-2 tokens to a fixed capacity C); the shared
expert is split 2 token-halves x 4 F-quarters (384 F-rows each) so its
weights stay tiny and SBUF-resident. Host does router + dispatch/combine
(the unshard step); device does all FLOPs-heavy matmuls.

v2: equal-size token chunks (every matmul N>=256 so LDWEIGHTS hides under
the moving stream), native SiLU on the ACT engine (one DVE mul instead of
two), one merged 3D-AP DMA per x tile / weight tensor / output tile, bf16
outputs (host upcasts), output DMAs issued on the idle sync queue.

Self-contained: hardcodes B=2,S=2048,D=768,E=8,K=2,F=1536.
"""
import ml_dtypes
import numpy as np
from contextlib import ExitStack

import concourse.bacc as bacc
import concourse.mybir as mybir
import concourse.tile as tile
from concourse.bass import ts
from concourse.bass_utils import run_bass_kernel_spmd

B, S, D = 2, 2048, 768
E, TOPK, F = 8, 2, 1536
T = B * S
NCORES = 8
KD = D // 128            # 6 contraction chunks over D
MF = F // 128            # 12 f-tiles for routed experts
MD = D // 128            # 6 output d-tiles
FS = 384                 # shared-expert F-slice per core (4 slices x 2 halves)
MFS = FS // 128          # 3 f-tiles for shared slice
TH = T // 2              # shared-expert token half
NT = 512                 # max moving-operand (token) tile

F32 = mybir.dt.float32
BF16 = mybir.dt.bfloat16
COMPUTE_DT = BF16
NP_COMPUTE = ml_dtypes.bfloat16

_cache: dict = {}


def _chunks(total, step=NT):
    """Split `total` into ceil(total/step) chunks, all multiples of 128,
    as equal as possible (keeps every matmul's moving dim >= 256)."""
    n = -(-total // step)
    per = -(-total // (n * 128)) * 128
    out, o = [], 0
    while o < total:
        c = min(per, total - o)
        out.append((o, c))
        o += c
    return out


def _build(C):
    """One SPMD program: routed expert over C tokens + shared slice over TH."""
    nc = bacc.Bacc("TRN2", debug=False)
    xeT = nc.dram_tensor("xeT", [D, C], COMPUTE_DT, kind="ExternalInput")
    wgT = nc.dram_tensor("wgT", [D, F], COMPUTE_DT, kind="ExternalInput")
    wuT = nc.dram_tensor("wuT", [D, F], COMPUTE_DT, kind="ExternalInput")
    wdT = nc.dram_tensor("wdT", [F, D], COMPUTE_DT, kind="ExternalInput")
    xsT = nc.dram_tensor("xsT", [D, TH], COMPUTE_DT, kind="ExternalInput")
    sgT = nc.dram_tensor("sgT", [D, FS], COMPUTE_DT, kind="ExternalInput")
    suT = nc.dram_tensor("suT", [D, FS], COMPUTE_DT, kind="ExternalInput")
    sdT = nc.dram_tensor("sdT", [FS, D], COMPUTE_DT, kind="ExternalInput")
    yeT = nc.dram_tensor("yeT", [D, C], BF16, kind="ExternalOutput")
    zT = nc.dram_tensor("zT", [D, TH], BF16, kind="ExternalOutput")

    with tile.TileContext(nc) as tc, ExitStack() as ctx:
        wpool = ctx.enter_context(tc.tile_pool(name="w", bufs=1))
        xpool = ctx.enter_context(tc.tile_pool(name="x", bufs=3))
        hpool = ctx.enter_context(tc.tile_pool(name="h", bufs=2))
        spool = ctx.enter_context(tc.tile_pool(name="s", bufs=3))
        opool = ctx.enter_context(tc.tile_pool(name="o", bufs=2))
        pgp = ctx.enter_context(tc.tile_pool(name="pg", bufs=2, space="PSUM"))
        pup = ctx.enter_context(tc.tile_pool(name="pu", bufs=2, space="PSUM"))
        pyp = ctx.enter_context(tc.tile_pool(name="py", bufs=3, space="PSUM"))

        def load_w(src, width, n, tag, fine=False):
            """One merged DMA: [n*128, width] DRAM -> [128, n*width] SBUF.
            fine=True issues per-k DMAs (deferred) so early matmuls only
            wait on the chunks they read."""
            t = wpool.tile([128, n * width], COMPUTE_DT, tag=tag)
            if fine:
                dmas = [lambda k=k: nc.sync.dma_start(
                    t[:, k * width:(k + 1) * width], src[ts(k, 128), :])
                    for k in range(n)]
                return t, dmas
            nc.sync.dma_start(
                t[:].rearrange("p (k j) -> p k j", k=n),
                src[:].rearrange("(k p) j -> p k j", p=128))
            return t

        # iteration list: shared-expert tiles FIRST (their weights are 8x
        # smaller, so the PE starts ~3.5us in while the big routed-expert
        # weights stream in behind).
        s_it = [("S", o, n) for o, n in _chunks(TH)]
        r_it = [("R", o, n) for o, n in _chunks(C)]
        iters = [s_it[0], s_it[1], r_it[0], s_it[2]] + \
                ([r_it[1]] if len(r_it) > 1 else []) + [s_it[3]] + r_it[2:]

        def load_x(ph, o, n, fine=False):
            """One merged DMA per token tile (POOL queue so x never FIFOs
            behind the big weight loads on the sync queue). fine=True splits
            per k-chunk so the first matmul can start sooner."""
            x_src = xeT if ph == "R" else xsT
            xt = xpool.tile([128, KD * NT], COMPUTE_DT, tag="xt")
            if fine:
                eng = nc.scalar if fine == "act" else nc.gpsimd
                for k in range(KD):
                    eng.dma_start(xt[:, k * NT:k * NT + n],
                                  x_src[ts(k, 128), o:o + n])
            else:
                nc.gpsimd.dma_start(
                    xt[:].rearrange("p (k j) -> p k j", k=KD)[:, :, :n],
                    x_src[:].rearrange("(k p) j -> p k j", p=128)[:, :, o:o + n])
            return xt

        # first x tiles before any weight DMA (nothing blocks the PE start);
        # fine-grained so the first matmul starts as soon as k-chunk 0 lands
        xts = [load_x(*iters[0], fine="act"), load_x(*iters[1], fine=True)]
        sg_sb, sg_dmas = load_w(sgT, FS, KD, "sg", fine=True)
        su_sb, su_dmas = load_w(suT, FS, KD, "su", fine=True)
        for dg, du in zip(sg_dmas, su_dmas):   # interleave gate/up chunks
            dg(); du()
        sd_sb = load_w(sdT, D, MFS, "sd")
        wg_sb = load_w(wgT, F, KD, "wg")
        wu_sb = load_w(wuT, F, KD, "wu")
        wd_sb = load_w(wdT, D, MF, "wd")

        def mm1(ph, o, n, xt):
            """gate/up matmuls + silu*up -> hT tiles for one iteration."""
            g_w, u_w, wid = (wg_sb, wu_sb, F) if ph == "R" else (sg_sb, su_sb, FS)
            mf = MF if ph == "R" else MFS
            hT = []
            for m in range(mf):
                g = pgp.tile([128, NT], F32, tag="pg")
                u = pup.tile([128, NT], F32, tag="pu")
                for k in range(KD):
                    nc.tensor.matmul(g[:, :n], g_w[:, k * wid + 128 * m:
                                                  k * wid + 128 * (m + 1)],
                                     xt[:, k * NT:k * NT + n],
                                     start=(k == 0), stop=(k == KD - 1))
                for k in range(KD):
                    nc.tensor.matmul(u[:, :n], u_w[:, k * wid + 128 * m:
                                                  k * wid + 128 * (m + 1)],
                                     xt[:, k * NT:k * NT + n],
                                     start=(k == 0), stop=(k == KD - 1))
                sil = spool.tile([128, NT], F32, tag="sil")
                nc.scalar.activation(sil[:, :n], g[:, :n],
                                     mybir.ActivationFunctionType.Silu)
                h = hpool.tile([128, NT], COMPUTE_DT, tag=f"h{m}")
                nc.vector.tensor_mul(h[:, :n], sil[:, :n], u[:, :n])
                hT.append(h)
            return hT

        def mm2(ph, o, n, hT, fine=False):
            """down-projection, copy-out, one merged output DMA per tile.
            fine=True DMAs per m2-slice so the final drain isn't serialized
            behind all six copies."""
            out_dst = yeT if ph == "R" else zT
            d_w = wd_sb if ph == "R" else sd_sb
            mf = MF if ph == "R" else MFS
            yo = opool.tile([128, MD * NT], BF16, tag="yo")
            for m2 in range(MD):
                y = pyp.tile([128, NT], F32, tag="py")
                for k2 in range(mf):
                    nc.tensor.matmul(y[:, :n], d_w[:, k2 * D + 128 * m2:
                                                  k2 * D + 128 * (m2 + 1)],
                                     hT[k2][:, :n],
                                     start=(k2 == 0), stop=(k2 == mf - 1))
                nc.scalar.copy(yo[:, m2 * NT:m2 * NT + n], y[:, :n])
                if fine:
                    nc.sync.dma_start(out_dst[ts(m2, 128), o:o + n],
                                      yo[:, m2 * NT:m2 * NT + n])
            if not fine:
                nc.sync.dma_start(
                    out_dst[:].rearrange("(m p) j -> p m j", p=128)[:, :, o:o + n],
                    yo[:].rearrange("p (m j) -> p m j", m=MD)[:, :, :n])

        # software pipeline: emit MM1(i+1) before MM2(i) so the PE chews on
        # the next tile's gate/up while ACT/DVE finish hT(i).
        hprev = None
        for i, it in enumerate(iters):
            if i + 2 < len(iters):
                xts.append(load_x(*iters[i + 2]))
            h = mm1(*it, xts[i])
            if hprev is not None:
                mm2(*iters[i - 1], hprev)
            hprev = h
        mm2(*iters[-1], hprev, fine=True)
    nc.compile()
    return nc


def _router(xf, w_router, expert_bias):
    """Replicates the reference router. f64 for stable top-k ordering,
    f32 softmax (same formula as jax.nn.softmax) for the weights."""
    logits = xf.astype(np.float64) @ w_router.T.astype(np.float64)
    l32 = (xf @ w_router.T).astype(np.float32)
    m = l32.max(-1, keepdims=True)
    e32 = np.exp(l32 - m)
    scores = e32 / e32.sum(-1, keepdims=True)
    e64 = np.exp(logits - logits.max(-1, keepdims=True))
    sel = e64 / e64.sum(-1, keepdims=True) + expert_bias.astype(np.float64)[None, :]
    top_idx = np.argsort(-sel, axis=-1, kind="stable")[:, :TOPK]
    top_s = np.take_along_axis(scores, top_idx, axis=-1)
    top_s = top_s / (top_s.sum(-1, keepdims=True) + 1e-9)
    return top_idx, top_s


def kernel(x, w_router, expert_bias, Wg, Wu, Wd, sg, su, sd):
    x = np.asarray(x); w_router = np.asarray(w_router)
    expert_bias = np.asarray(expert_bias)
    Wg = np.asarray(Wg); Wu = np.asarray(Wu); Wd = np.asarray(Wd)
    sg = np.asarray(sg); su = np.asarray(su); sd = np.asarray(sd)
    xf = x.reshape(-1, D).astype(np.float32)

    top_idx, top_s = _router(xf, w_router, expert_bias)

    idxs, ws = [], []
    for e in range(E):
        hit = (top_idx == e)
        tok = np.nonzero(hit.any(-1))[0]
        idxs.append(tok)
        ws.append(top_s[tok][hit[tok]])
    cmax = max(len(i) for i in idxs)
    C = max(512, -(-cmax // 128) * 128)

    key = C
    if key not in _cache:
        _cache[key] = _build(C)
    nc = _cache[key]

    cast = lambda a: np.ascontiguousarray(a, dtype=np.float32).astype(NP_COMPUTE)
    in_maps = []
    for e in range(E):
        xeT = np.zeros((D, C), np.float32)
        xeT[:, :len(idxs[e])] = xf[idxs[e]].T
        th, fq = e // 4, e % 4
        in_maps.append({
            "xeT": cast(xeT),
            "wgT": cast(Wg[e].T), "wuT": cast(Wu[e].T), "wdT": cast(Wd[e].T),
            "xsT": cast(xf[th * TH:(th + 1) * TH].T),
            "sgT": cast(sg[fq * FS:(fq + 1) * FS].T),
            "suT": cast(su[fq * FS:(fq + 1) * FS].T),
            "sdT": cast(sd[:, fq * FS:(fq + 1) * FS].T),
        })

    res = run_bass_kernel_spmd(nc, in_maps, core_ids=list(range(NCORES)))

    out = np.zeros((T, D), np.float32)
    for e in range(E):
        ye = res.results[e]["yeT"].astype(np.float32).T[:len(idxs[e])]
        out[idxs[e]] += ws[e][:, None] * ye
        th = e // 4
        out[th * TH:(th + 1) * TH] += res.results[e]["zT"].astype(np.float32).T
    return out.reshape(B, S, D).astype(x.dtype)


# revision 17
# speedup vs baseline: 42.4180x; 9.3242x over previous
"""MoE FFN (DeepSeek-style top-2 routing + shared expert) on 8 TRN2 cores.

Sharding: expert-parallel for the 8 routed experts (core e owns expert e,
host gathers/pads its top-2 tokens to a fixed capacity C); the shared
expert is split 2 token-halves x 4 F-quarters (384 F-rows each) so its
weights stay tiny and SBUF-resident. Host does router + dispatch/combine
(the unshard step); device does all FLOPs-heavy matmuls.

v2: native SiLU on the ACT engine (one DVE mul instead of two), one merged
3D-AP DMA per x tile / weight tensor / output tile, bf16 outputs (host
upcasts), output DMAs issued on the idle sync queue, fine-grained first
loads so the PE starts ~2us in. Greedy [512,...,tail] token chunks
(paired-slope A/B on silicon beat equal-size chunks by ~15us/exec).

Self-contained: hardcodes B=2,S=2048,D=768,E=8,K=2,F=1536.
"""
import ml_dtypes
import numpy as np
from contextlib import ExitStack

import concourse.bacc as bacc
import concourse.mybir as mybir
import concourse.tile as tile
from concourse.bass import ts
from concourse.bass_utils import run_bass_kernel_spmd

B, S, D = 2, 2048, 768
E, TOPK, F = 8, 2, 1536
T = B * S
NCORES = 8
KD = D // 128            # 6 contraction chunks over D
MF = F // 128            # 12 f-tiles for routed experts
MD = D // 128            # 6 output d-tiles
FS = 384                 # shared-expert F-slice per core (4 slices x 2 halves)
MFS = FS // 128          # 3 f-tiles for shared slice
TH = T // 2              # shared-expert token half
NT = 512                 # max moving-operand (token) tile

F32 = mybir.dt.float32
BF16 = mybir.dt.bfloat16
COMPUTE_DT = BF16
NP_COMPUTE = ml_dtypes.bfloat16

_cache: dict = {}


def _chunks(total, step=NT):
    """Greedy split [512, ..., tail]: measured faster on silicon than
    equal-size chunks (paired slope A/B, ~15us/exec)."""
    out, o = [], 0
    while o < total:
        c = min(step, total - o)
        out.append((o, c))
        o += c
    return out


def _build(C, reps=1):
    """One SPMD program: routed expert over C tokens + shared slice over TH.
    reps>1 repeats the whole computation back-to-back (timing experiments
    only -- slope of wall time vs reps isolates per-exec device time)."""
    nc = bacc.Bacc("TRN2", debug=False)
    xeT = nc.dram_tensor("xeT", [D, C], COMPUTE_DT, kind="ExternalInput")
    wgT = nc.dram_tensor("wgT", [D, F], COMPUTE_DT, kind="ExternalInput")
    wuT = nc.dram_tensor("wuT", [D, F], COMPUTE_DT, kind="ExternalInput")
    wdT = nc.dram_tensor("wdT", [F, D], COMPUTE_DT, kind="ExternalInput")
    xsT = nc.dram_tensor("xsT", [D, TH], COMPUTE_DT, kind="ExternalInput")
    sgT = nc.dram_tensor("sgT", [D, FS], COMPUTE_DT, kind="ExternalInput")
    suT = nc.dram_tensor("suT", [D, FS], COMPUTE_DT, kind="ExternalInput")
    sdT = nc.dram_tensor("sdT", [FS, D], COMPUTE_DT, kind="ExternalInput")
    yeT = nc.dram_tensor("yeT", [D, C], BF16, kind="ExternalOutput")
    zT = nc.dram_tensor("zT", [D, TH], BF16, kind="ExternalOutput")

    with tile.TileContext(nc) as tc, ExitStack() as ctx:
        wpool = ctx.enter_context(tc.tile_pool(name="w", bufs=1))
        xpool = ctx.enter_context(tc.tile_pool(name="x", bufs=3))
        hpool = ctx.enter_context(tc.tile_pool(name="h", bufs=2))
        spool = ctx.enter_context(tc.tile_pool(name="s", bufs=3))
        opool = ctx.enter_context(tc.tile_pool(name="o", bufs=2))
        pgp = ctx.enter_context(tc.tile_pool(name="pg", bufs=2, space="PSUM"))
        pup = ctx.enter_context(tc.tile_pool(name="pu", bufs=2, space="PSUM"))
        pyp = ctx.enter_context(tc.tile_pool(name="py", bufs=3, space="PSUM"))

        def load_w(src, width, n, tag, fine=False):
            """One merged DMA: [n*128, width] DRAM -> [128, n*width] SBUF.
            fine=True issues per-k DMAs (deferred) so early matmuls only
            wait on the chunks they read."""
            t = wpool.tile([128, n * width], COMPUTE_DT, tag=tag)
            if fine:
                dmas = [lambda k=k: nc.sync.dma_start(
                    t[:, k * width:(k + 1) * width], src[ts(k, 128), :])
                    for k in range(n)]
                return t, dmas
            nc.sync.dma_start(
                t[:].rearrange("p (k j) -> p k j", k=n),
                src[:].rearrange("(k p) j -> p k j", p=128))
            return t

        # iteration list: shared-expert tiles FIRST (their weights are 8x
        # smaller, so the PE starts ~3.5us in while the big routed-expert
        # weights stream in behind).
        s_it = [("S", o, n) for o, n in _chunks(TH)]
        r_it = [("R", o, n) for o, n in _chunks(C)]
        iters = [s_it[0], s_it[1], r_it[0], s_it[2]] + \
                ([r_it[1]] if len(r_it) > 1 else []) + [s_it[3]] + r_it[2:]
        iters = iters * reps

        def load_x(ph, o, n, fine=False):
            """One merged DMA per token tile (POOL queue so x never FIFOs
            behind the big weight loads on the sync queue). fine=True splits
            per k-chunk so the first matmul can start sooner."""
            x_src = xeT if ph == "R" else xsT
            xt = xpool.tile([128, KD * NT], COMPUTE_DT, tag="xt")
            if fine:
                eng = nc.scalar if fine == "act" else nc.gpsimd
                for k in range(KD):
                    eng.dma_start(xt[:, k * NT:k * NT + n],
                                  x_src[ts(k, 128), o:o + n])
            else:
                nc.gpsimd.dma_start(
                    xt[:].rearrange("p (k j) -> p k j", k=KD)[:, :, :n],
                    x_src[:].rearrange("(k p) j -> p k j", p=128)[:, :, o:o + n])
            return xt

        # first x tiles before any weight DMA (nothing blocks the PE start);
        # fine-grained so the first matmul starts as soon as k-chunk 0 lands
        xts = [load_x(*iters[0], fine="act"), load_x(*iters[1], fine=True)]
        sg_sb, sg_dmas = load_w(sgT, FS, KD, "sg", fine=True)
        su_sb, su_dmas = load_w(suT, FS, KD, "su", fine=True)
        for dg, du in zip(sg_dmas, su_dmas):   # interleave gate/up chunks
            dg(); du()
        sd_sb = load_w(sdT, D, MFS, "sd")
        wg_sb = load_w(wgT, F, KD, "wg")
        wu_sb = load_w(wuT, F, KD, "wu")
        wd_sb = load_w(wdT, D, MF, "wd")

        def mm1(ph, o, n, xt):
            """gate/up matmuls + silu*up -> hT tiles for one iteration."""
            g_w, u_w, wid = (wg_sb, wu_sb, F) if ph == "R" else (sg_sb, su_sb, FS)
            mf = MF if ph == "R" else MFS
            hT = []
            for m in range(mf):
                g = pgp.tile([128, NT], F32, tag="pg")
                u = pup.tile([128, NT], F32, tag="pu")
                for k in range(KD):
                    nc.tensor.matmul(g[:, :n], g_w[:, k * wid + 128 * m:
                                                  k * wid + 128 * (m + 1)],
                                     xt[:, k * NT:k * NT + n],
                                     start=(k == 0), stop=(k == KD - 1))
                for k in range(KD):
                    nc.tensor.matmul(u[:, :n], u_w[:, k * wid + 128 * m:
                                                  k * wid + 128 * (m + 1)],
                                     xt[:, k * NT:k * NT + n],
                                     start=(k == 0), stop=(k == KD - 1))
                sil = spool.tile([128, NT], F32, tag="sil")
                nc.scalar.activation(sil[:, :n], g[:, :n],
                                     mybir.ActivationFunctionType.Silu)
                h = hpool.tile([128, NT], COMPUTE_DT, tag=f"h{m}")
                nc.vector.tensor_mul(h[:, :n], sil[:, :n], u[:, :n])
                hT.append(h)
            return hT

        def mm2(ph, o, n, hT, fine=False):
            """down-projection, copy-out, one merged output DMA per tile.
            fine=True DMAs per m2-slice so the final drain isn't serialized
            behind all six copies."""
            out_dst = yeT if ph == "R" else zT
            d_w = wd_sb if ph == "R" else sd_sb
            mf = MF if ph == "R" else MFS
            yo = opool.tile([128, MD * NT], BF16, tag="yo")
            for m2 in range(MD):
                y = pyp.tile([128, NT], F32, tag="py")
                for k2 in range(mf):
                    nc.tensor.matmul(y[:, :n], d_w[:, k2 * D + 128 * m2:
                                                  k2 * D + 128 * (m2 + 1)],
                                     hT[k2][:, :n],
                                     start=(k2 == 0), stop=(k2 == mf - 1))
                nc.scalar.copy(yo[:, m2 * NT:m2 * NT + n], y[:, :n])
                if fine:
                    nc.sync.dma_start(out_dst[ts(m2, 128), o:o + n],
                                      yo[:, m2 * NT:m2 * NT + n])
            if not fine:
                nc.sync.dma_start(
                    out_dst[:].rearrange("(m p) j -> p m j", p=128)[:, :, o:o + n],
                    yo[:].rearrange("p (m j) -> p m j", m=MD)[:, :, :n])

        # software pipeline: emit MM1(i+1) before MM2(i) so the PE chews on
        # the next tile's gate/up while ACT/DVE finish hT(i).
        hprev = None
        for i, it in enumerate(iters):
            if i + 2 < len(iters):
                xts.append(load_x(*iters[i + 2]))
            h = mm1(*it, xts[i])
            if hprev is not None:
                mm2(*iters[i - 1], hprev)
            hprev = h
        mm2(*iters[-1], hprev, fine=True)
    nc.compile()
    return nc


def _router(xf, w_router, expert_bias):
    """Replicates the reference router. f64 for stable top-k ordering,
    f32 softmax (same formula as jax.nn.softmax) for the weights."""
    logits = xf.astype(np.float64) @ w_router.T.astype(np.float64)
    l32 = (xf @ w_router.T).astype(np.float32)
    m = l32.max(-1, keepdims=True)
    e32 = np.exp(l32 - m)
    scores = e32 / e32.sum(-1, keepdims=True)
    e64 = np.exp(logits - logits.max(-1, keepdims=True))
    sel = e64 / e64.sum(-1, keepdims=True) + expert_bias.astype(np.float64)[None, :]
    top_idx = np.argsort(-sel, axis=-1, kind="stable")[:, :TOPK]
    top_s = np.take_along_axis(scores, top_idx, axis=-1)
    top_s = top_s / (top_s.sum(-1, keepdims=True) + 1e-9)
    return top_idx, top_s


def kernel(x, w_router, expert_bias, Wg, Wu, Wd, sg, su, sd):
    x = np.asarray(x); w_router = np.asarray(w_router)
    expert_bias = np.asarray(expert_bias)
    Wg = np.asarray(Wg); Wu = np.asarray(Wu); Wd = np.asarray(Wd)
    sg = np.asarray(sg); su = np.asarray(su); sd = np.asarray(sd)
    xf = x.reshape(-1, D).astype(np.float32)

    top_idx, top_s = _router(xf, w_router, expert_bias)

    idxs, ws = [], []
    for e in range(E):
        hit = (top_idx == e)
        tok = np.nonzero(hit.any(-1))[0]
        idxs.append(tok)
        ws.append(top_s[tok][hit[tok]])
    cmax = max(len(i) for i in idxs)
    C = max(512, -(-cmax // 128) * 128)

    key = C
    if key not in _cache:
        _cache[key] = _build(C)
    nc = _cache[key]

    cast = lambda a: np.ascontiguousarray(a, dtype=np.float32).astype(NP_COMPUTE)
    in_maps = []
    for e in range(E):
        xeT = np.zeros((D, C), np.float32)
        xeT[:, :len(idxs[e])] = xf[idxs[e]].T
        th, fq = e // 4, e % 4
        in_maps.append({
            "xeT": cast(xeT),
            "wgT": cast(Wg[e].T), "wuT": cast(Wu[e].T), "wdT": cast(Wd[e].T),
            "xsT": cast(xf[th * TH:(th + 1) * TH].T),
            "sgT": cast(sg[fq * FS:(fq + 1) * FS].T),
            "suT": cast(su[fq * FS:(fq + 1) * FS].T),
            "sdT": cast(sd[:, fq * FS:(fq + 1) * FS].T),
        })

    res = run_bass_kernel_spmd(nc, in_maps, core_ids=list(range(NCORES)))

    out = np.zeros((T, D), np.float32)
    for e in range(E):
        ye = res.results[e]["yeT"].astype(np.float32).T[:len(idxs[e])]
        out[idxs[e]] += ws[e][:, None] * ye
        th = e // 4
        out[th * TH:(th + 1) * TH] += res.results[e]["zT"].astype(np.float32).T
    return out.reshape(B, S, D).astype(x.dtype)


# revision 19
# speedup vs baseline: 204.4479x; 4.8198x over previous
"""MoE FFN (DeepSeek-style top-2 routing + shared expert) on 8 TRN2 cores.

Sharding: expert-parallel for the 8 routed experts (core e owns expert e,
host gathers/pads its top-2 tokens to a fixed capacity C); the shared
expert is split 2 token-halves x 4 F-quarters (384 F-rows each) so its
weights stay tiny and SBUF-resident. Host does router + dispatch/combine
(the unshard step); device does all FLOPs-heavy matmuls.

v3: native SiLU on the ACT engine (one DVE mul instead of two), one merged
3D-AP DMA per x tile / weight tensor / output tile, bf16 outputs (host
upcasts), output DMAs issued on the idle sync queue, fine-grained first
loads so the PE starts ~2us in. Token chunks greedy [512,...] with the
tail kept >= 256: paired-slope A/B on silicon measured ~147us/exec vs
183us for a 128-wide tail and 181us for equal-384 chunks.

Self-contained: hardcodes B=2,S=2048,D=768,E=8,K=2,F=1536.
"""
import ml_dtypes
import numpy as np
from contextlib import ExitStack

import concourse.bacc as bacc
import concourse.mybir as mybir
import concourse.tile as tile
from concourse.bass import ts
from concourse.bass_utils import run_bass_kernel_spmd

B, S, D = 2, 2048, 768
E, TOPK, F = 8, 2, 1536
T = B * S
NCORES = 8
KD = D // 128            # 6 contraction chunks over D
MF = F // 128            # 12 f-tiles for routed experts
MD = D // 128            # 6 output d-tiles
FS = 384                 # shared-expert F-slice per core (4 slices x 2 halves)
MFS = FS // 128          # 3 f-tiles for shared slice
TH = T // 2              # shared-expert token half
NT = 512                 # max moving-operand (token) tile

F32 = mybir.dt.float32
BF16 = mybir.dt.bfloat16
COMPUTE_DT = BF16
NP_COMPUTE = ml_dtypes.bfloat16

_cache: dict = {}


def _chunks(total, step=NT):
    """Greedy split [512, ..., tail], but keep the tail >= 256 (steal 128
    from the previous chunk): N=128 tiles measured ~35us/exec slower on
    silicon (exposed LDWEIGHTS + thin-tile PE idling the cost model does
    not charge)."""
    out, o = [], 0
    while o < total:
        c = min(step, total - o)
        out.append((o, c))
        o += c
    if len(out) > 1 and out[-1][1] < 256:
        o_prev, c_prev = out[-2]
        o_last, c_last = out[-1]
        out[-2] = (o_prev, c_prev - 128)
        out[-1] = (o_last - 128, c_last + 128)
    return out


def _build(C, reps=1):
    """One SPMD program: routed expert over C tokens + shared slice over TH.
    reps>1 repeats the whole computation back-to-back (timing experiments
    only -- slope of wall time vs reps isolates per-exec device time)."""
    nc = bacc.Bacc("TRN2", debug=False)
    xeT = nc.dram_tensor("xeT", [D, C], COMPUTE_DT, kind="ExternalInput")
    wgT = nc.dram_tensor("wgT", [D, F], COMPUTE_DT, kind="ExternalInput")
    wuT = nc.dram_tensor("wuT", [D, F], COMPUTE_DT, kind="ExternalInput")
    wdT = nc.dram_tensor("wdT", [F, D], COMPUTE_DT, kind="ExternalInput")
    xsT = nc.dram_tensor("xsT", [D, TH], COMPUTE_DT, kind="ExternalInput")
    sgT = nc.dram_tensor("sgT", [D, FS], COMPUTE_DT, kind="ExternalInput")
    suT = nc.dram_tensor("suT", [D, FS], COMPUTE_DT, kind="ExternalInput")
    sdT = nc.dram_tensor("sdT", [FS, D], COMPUTE_DT, kind="ExternalInput")
    yeT = nc.dram_tensor("yeT", [D, C], BF16, kind="ExternalOutput")
    zT = nc.dram_tensor("zT", [D, TH], BF16, kind="ExternalOutput")

    with tile.TileContext(nc) as tc, ExitStack() as ctx:
        wpool = ctx.enter_context(tc.tile_pool(name="w", bufs=1))
        xpool = ctx.enter_context(tc.tile_pool(name="x", bufs=3))
        hpool = ctx.enter_context(tc.tile_pool(name="h", bufs=2))
        spool = ctx.enter_context(tc.tile_pool(name="s", bufs=3))
        opool = ctx.enter_context(tc.tile_pool(name="o", bufs=2))
        pgp = ctx.enter_context(tc.tile_pool(name="pg", bufs=2, space="PSUM"))
        pup = ctx.enter_context(tc.tile_pool(name="pu", bufs=2, space="PSUM"))
        pyp = ctx.enter_context(tc.tile_pool(name="py", bufs=3, space="PSUM"))

        def load_w(src, width, n, tag, fine=False):
            """One merged DMA: [n*128, width] DRAM -> [128, n*width] SBUF.
            fine=True issues per-k DMAs (deferred) so early matmuls only
            wait on the chunks they read."""
            t = wpool.tile([128, n * width], COMPUTE_DT, tag=tag)
            if fine:
                dmas = [lambda k=k: nc.sync.dma_start(
                    t[:, k * width:(k + 1) * width], src[ts(k, 128), :])
                    for k in range(n)]
                return t, dmas
            nc.sync.dma_start(
                t[:].rearrange("p (k j) -> p k j", k=n),
                src[:].rearrange("(k p) j -> p k j", p=128))
            return t

        # iteration list: shared-expert tiles FIRST (their weights are 8x
        # smaller, so the PE starts ~3.5us in while the big routed-expert
        # weights stream in behind).
        s_it = [("S", o, n) for o, n in _chunks(TH)]
        r_it = [("R", o, n) for o, n in _chunks(C)]
        iters = [s_it[0], s_it[1], r_it[0], s_it[2]] + \
                ([r_it[1]] if len(r_it) > 1 else []) + [s_it[3]] + r_it[2:]
        iters = iters * reps

        def load_x(ph, o, n, fine=False):
            """One merged DMA per token tile (POOL queue so x never FIFOs
            behind the big weight loads on the sync queue). fine=True splits
            per k-chunk so the first matmul can start sooner."""
            x_src = xeT if ph == "R" else xsT
            xt = xpool.tile([128, KD * NT], COMPUTE_DT, tag="xt")
            if fine:
                eng = nc.scalar if fine == "act" else nc.gpsimd
                for k in range(KD):
                    eng.dma_start(xt[:, k * NT:k * NT + n],
                                  x_src[ts(k, 128), o:o + n])
            else:
                nc.gpsimd.dma_start(
                    xt[:].rearrange("p (k j) -> p k j", k=KD)[:, :, :n],
                    x_src[:].rearrange("(k p) j -> p k j", p=128)[:, :, o:o + n])
            return xt

        # first x tiles before any weight DMA (nothing blocks the PE start);
        # fine-grained so the first matmul starts as soon as k-chunk 0 lands
        xts = [load_x(*iters[0], fine="act"), load_x(*iters[1], fine=True)]
        sg_sb, sg_dmas = load_w(sgT, FS, KD, "sg", fine=True)
        su_sb, su_dmas = load_w(suT, FS, KD, "su", fine=True)
        for dg, du in zip(sg_dmas, su_dmas):   # interleave gate/up chunks
            dg(); du()
        sd_sb = load_w(sdT, D, MFS, "sd")
        wg_sb = load_w(wgT, F, KD, "wg")
        wu_sb = load_w(wuT, F, KD, "wu")
        wd_sb = load_w(wdT, D, MF, "wd")

        def mm1(ph, o, n, xt):
            """gate/up matmuls + silu*up -> hT tiles for one iteration."""
            g_w, u_w, wid = (wg_sb, wu_sb, F) if ph == "R" else (sg_sb, su_sb, FS)
            mf = MF if ph == "R" else MFS
            hT = []
            for m in range(mf):
                g = pgp.tile([128, NT], F32, tag="pg")
                u = pup.tile([128, NT], F32, tag="pu")
                for k in range(KD):
                    nc.tensor.matmul(g[:, :n], g_w[:, k * wid + 128 * m:
                                                  k * wid + 128 * (m + 1)],
                                     xt[:, k * NT:k * NT + n],
                                     start=(k == 0), stop=(k == KD - 1))
                for k in range(KD):
                    nc.tensor.matmul(u[:, :n], u_w[:, k * wid + 128 * m:
                                                  k * wid + 128 * (m + 1)],
                                     xt[:, k * NT:k * NT + n],
                                     start=(k == 0), stop=(k == KD - 1))
                sil = spool.tile([128, NT], F32, tag="sil")
                nc.scalar.activation(sil[:, :n], g[:, :n],
                                     mybir.ActivationFunctionType.Silu)
                h = hpool.tile([128, NT], COMPUTE_DT, tag=f"h{m}")
                nc.vector.tensor_mul(h[:, :n], sil[:, :n], u[:, :n])
                hT.append(h)
            return hT

        def mm2(ph, o, n, hT, fine=False):
            """down-projection, copy-out, one merged output DMA per tile.
            fine=True DMAs per m2-slice so the final drain isn't serialized
            behind all six copies."""
            out_dst = yeT if ph == "R" else zT
            d_w = wd_sb if ph == "R" else sd_sb
            mf = MF if ph == "R" else MFS
            yo = opool.tile([128, MD * NT], BF16, tag="yo")
            for m2 in range(MD):
                y = pyp.tile([128, NT], F32, tag="py")
                for k2 in range(mf):
                    nc.tensor.matmul(y[:, :n], d_w[:, k2 * D + 128 * m2:
                                                  k2 * D + 128 * (m2 + 1)],
                                     hT[k2][:, :n],
                                     start=(k2 == 0), stop=(k2 == mf - 1))
                nc.scalar.copy(yo[:, m2 * NT:m2 * NT + n], y[:, :n])
                if fine:
                    nc.sync.dma_start(out_dst[ts(m2, 128), o:o + n],
                                      yo[:, m2 * NT:m2 * NT + n])
            if not fine:
                nc.sync.dma_start(
                    out_dst[:].rearrange("(m p) j -> p m j", p=128)[:, :, o:o + n],
                    yo[:].rearrange("p (m j) -> p m j", m=MD)[:, :, :n])

        # software pipeline: emit MM1(i+1) before MM2(i) so the PE chews on
        # the next tile's gate/up while ACT/DVE finish hT(i).
        hprev = None
        for i, it in enumerate(iters):
            if i + 2 < len(iters):
                xts.append(load_x(*iters[i + 2]))
            h = mm1(*it, xts[i])
            if hprev is not None:
                mm2(*iters[i - 1], hprev)
            hprev = h
        mm2(*iters[-1], hprev, fine=True)
    nc.compile()
    return nc


def _router(xf, w_router, expert_bias):
    """Replicates the reference router. f64 for stable top-k ordering,
    f32 softmax (same formula as jax.nn.softmax) for the weights."""
    logits = xf.astype(np.float64) @ w_router.T.astype(np.float64)
    l32 = (xf @ w_router.T).astype(np.float32)
    m = l32.max(-1, keepdims=True)
    e32 = np.exp(l32 - m)
    scores = e32 / e32.sum(-1, keepdims=True)
    e64 = np.exp(logits - logits.max(-1, keepdims=True))
    sel = e64 / e64.sum(-1, keepdims=True) + expert_bias.astype(np.float64)[None, :]
    top_idx = np.argsort(-sel, axis=-1, kind="stable")[:, :TOPK]
    top_s = np.take_along_axis(scores, top_idx, axis=-1)
    top_s = top_s / (top_s.sum(-1, keepdims=True) + 1e-9)
    return top_idx, top_s


def kernel(x, w_router, expert_bias, Wg, Wu, Wd, sg, su, sd):
    x = np.asarray(x); w_router = np.asarray(w_router)
    expert_bias = np.asarray(expert_bias)
    Wg = np.asarray(Wg); Wu = np.asarray(Wu); Wd = np.asarray(Wd)
    sg = np.asarray(sg); su = np.asarray(su); sd = np.asarray(sd)
    xf = x.reshape(-1, D).astype(np.float32)

    top_idx, top_s = _router(xf, w_router, expert_bias)

    idxs, ws = [], []
    for e in range(E):
        hit = (top_idx == e)
        tok = np.nonzero(hit.any(-1))[0]
        idxs.append(tok)
        ws.append(top_s[tok][hit[tok]])
    cmax = max(len(i) for i in idxs)
    C = max(512, -(-cmax // 128) * 128)

    key = C
    if key not in _cache:
        _cache[key] = _build(C)
    nc = _cache[key]

    cast = lambda a: np.ascontiguousarray(a, dtype=np.float32).astype(NP_COMPUTE)
    in_maps = []
    for e in range(E):
        xeT = np.zeros((D, C), np.float32)
        xeT[:, :len(idxs[e])] = xf[idxs[e]].T
        th, fq = e // 4, e % 4
        in_maps.append({
            "xeT": cast(xeT),
            "wgT": cast(Wg[e].T), "wuT": cast(Wu[e].T), "wdT": cast(Wd[e].T),
            "xsT": cast(xf[th * TH:(th + 1) * TH].T),
            "sgT": cast(sg[fq * FS:(fq + 1) * FS].T),
            "suT": cast(su[fq * FS:(fq + 1) * FS].T),
            "sdT": cast(sd[:, fq * FS:(fq + 1) * FS].T),
        })

    res = run_bass_kernel_spmd(nc, in_maps, core_ids=list(range(NCORES)))

    out = np.zeros((T, D), np.float32)
    for e in range(E):
        ye = res.results[e]["yeT"].astype(np.float32).T[:len(idxs[e])]
        out[idxs[e]] += ws[e][:, None] * ye
        th = e // 4
        out[th * TH:(th + 1) * TH] += res.results[e]["zT"].astype(np.float32).T
    return out.reshape(B, S, D).astype(x.dtype)


# revision 21
# speedup vs baseline: 645.1631x; 3.1556x over previous
"""MoE FFN (DeepSeek-style top-2 routing + shared expert) on 8 TRN2 cores.

Sharding: expert-parallel for the 8 routed experts (core e owns expert e,
host gathers/pads its top-2 tokens to a fixed capacity C); the shared
expert is split 2 token-halves x 4 F-quarters (384 F-rows each) so its
weights stay tiny and SBUF-resident. Host does router + dispatch/combine
(the unshard step); device does all FLOPs-heavy matmuls.

v5: native SiLU on the ACT engine (one DVE mul instead of two); mm2
output copies alternate DVE/ACT per m2 so neither strict-FIFO queue
stacks copies in front of critical-path silu/mul (paired A/B: alternate
146.5 < DVE-all 184.4; DVE-all 137.9 < ACT-all 171.9), one merged
3D-AP DMA per x tile / weight tensor / output tile, bf16 outputs (host
upcasts), output DMAs issued on the idle sync queue, fine-grained first
loads so the PE starts ~2us in. Token chunks greedy [512,...] with the
tail kept >= 256: paired-slope A/B on silicon measured ~147us/exec vs
183us for a 128-wide tail and 181us for equal-384 chunks.

Self-contained: hardcodes B=2,S=2048,D=768,E=8,K=2,F=1536.
"""
import ml_dtypes
import numpy as np
from contextlib import ExitStack

import concourse.bacc as bacc
import concourse.mybir as mybir
import concourse.tile as tile
from concourse.bass import ts
from concourse.bass_utils import run_bass_kernel_spmd

B, S, D = 2, 2048, 768
E, TOPK, F = 8, 2, 1536
T = B * S
NCORES = 8
KD = D // 128            # 6 contraction chunks over D
MF = F // 128            # 12 f-tiles for routed experts
MD = D // 128            # 6 output d-tiles
FS = 384                 # shared-expert F-slice per core (4 slices x 2 halves)
MFS = FS // 128          # 3 f-tiles for shared slice
TH = T // 2              # shared-expert token half
NT = 512                 # max moving-operand (token) tile

F32 = mybir.dt.float32
BF16 = mybir.dt.bfloat16
COMPUTE_DT = BF16
NP_COMPUTE = ml_dtypes.bfloat16

_cache: dict = {}


def _chunks(total, step=NT):
    """Greedy split [512, ..., tail], but keep the tail >= 256 (steal 128
    from the previous chunk): N=128 tiles measured ~35us/exec slower on
    silicon (exposed LDWEIGHTS + thin-tile PE idling the cost model does
    not charge)."""
    out, o = [], 0
    while o < total:
        c = min(step, total - o)
        out.append((o, c))
        o += c
    if len(out) > 1 and out[-1][1] < 256:
        o_prev, c_prev = out[-2]
        o_last, c_last = out[-1]
        out[-2] = (o_prev, c_prev - 128)
        out[-1] = (o_last - 128, c_last + 128)
    return out


def _build(C, reps=1):
    """One SPMD program: routed expert over C tokens + shared slice over TH.
    reps>1 repeats the whole computation back-to-back (timing experiments
    only -- slope of wall time vs reps isolates per-exec device time)."""
    nc = bacc.Bacc("TRN2", debug=False)
    xeT = nc.dram_tensor("xeT", [D, C], COMPUTE_DT, kind="ExternalInput")
    wgT = nc.dram_tensor("wgT", [D, F], COMPUTE_DT, kind="ExternalInput")
    wuT = nc.dram_tensor("wuT", [D, F], COMPUTE_DT, kind="ExternalInput")
    wdT = nc.dram_tensor("wdT", [F, D], COMPUTE_DT, kind="ExternalInput")
    xsT = nc.dram_tensor("xsT", [D, TH], COMPUTE_DT, kind="ExternalInput")
    sgT = nc.dram_tensor("sgT", [D, FS], COMPUTE_DT, kind="ExternalInput")
    suT = nc.dram_tensor("suT", [D, FS], COMPUTE_DT, kind="ExternalInput")
    sdT = nc.dram_tensor("sdT", [FS, D], COMPUTE_DT, kind="ExternalInput")
    yeT = nc.dram_tensor("yeT", [D, C], BF16, kind="ExternalOutput")
    zT = nc.dram_tensor("zT", [D, TH], BF16, kind="ExternalOutput")

    with tile.TileContext(nc) as tc, ExitStack() as ctx:
        wpool = ctx.enter_context(tc.tile_pool(name="w", bufs=1))
        xpool = ctx.enter_context(tc.tile_pool(name="x", bufs=3))
        hpool = ctx.enter_context(tc.tile_pool(name="h", bufs=2))
        spool = ctx.enter_context(tc.tile_pool(name="s", bufs=3))
        opool = ctx.enter_context(tc.tile_pool(name="o", bufs=2))
        pgp = ctx.enter_context(tc.tile_pool(name="pg", bufs=2, space="PSUM"))
        pup = ctx.enter_context(tc.tile_pool(name="pu", bufs=2, space="PSUM"))
        pyp = ctx.enter_context(tc.tile_pool(name="py", bufs=3, space="PSUM"))

        def load_w(src, width, n, tag, fine=False):
            """One merged DMA: [n*128, width] DRAM -> [128, n*width] SBUF.
            fine=True issues per-k DMAs (deferred) so early matmuls only
            wait on the chunks they read."""
            t = wpool.tile([128, n * width], COMPUTE_DT, tag=tag)
            if fine:
                dmas = [lambda k=k: nc.sync.dma_start(
                    t[:, k * width:(k + 1) * width], src[ts(k, 128), :])
                    for k in range(n)]
                return t, dmas
            nc.sync.dma_start(
                t[:].rearrange("p (k j) -> p k j", k=n),
                src[:].rearrange("(k p) j -> p k j", p=128))
            return t

        # iteration list: shared-expert tiles FIRST (their weights are 8x
        # smaller, so the PE starts ~3.5us in while the big routed-expert
        # weights stream in behind).
        s_it = [("S", o, n) for o, n in _chunks(TH)]
        r_it = [("R", o, n) for o, n in _chunks(C)]
        iters = [s_it[0], s_it[1], r_it[0], s_it[2]] + \
                ([r_it[1]] if len(r_it) > 1 else []) + [s_it[3]] + r_it[2:]
        iters = iters * reps

        def load_x(ph, o, n, fine=False):
            """One merged DMA per token tile (POOL queue so x never FIFOs
            behind the big weight loads on the sync queue). fine=True splits
            per k-chunk so the first matmul can start sooner."""
            x_src = xeT if ph == "R" else xsT
            xt = xpool.tile([128, KD * NT], COMPUTE_DT, tag="xt")
            if fine:
                eng = nc.scalar if fine == "act" else nc.gpsimd
                for k in range(KD):
                    eng.dma_start(xt[:, k * NT:k * NT + n],
                                  x_src[ts(k, 128), o:o + n])
            else:
                nc.gpsimd.dma_start(
                    xt[:].rearrange("p (k j) -> p k j", k=KD)[:, :, :n],
                    x_src[:].rearrange("(k p) j -> p k j", p=128)[:, :, o:o + n])
            return xt

        # first x tiles before any weight DMA (nothing blocks the PE start);
        # fine-grained so the first matmul starts as soon as k-chunk 0 lands
        xts = [load_x(*iters[0], fine="act"), load_x(*iters[1], fine=True)]
        sg_sb, sg_dmas = load_w(sgT, FS, KD, "sg", fine=True)
        su_sb, su_dmas = load_w(suT, FS, KD, "su", fine=True)
        for dg, du in zip(sg_dmas, su_dmas):   # interleave gate/up chunks
            dg(); du()
        sd_sb = load_w(sdT, D, MFS, "sd")
        wg_sb = load_w(wgT, F, KD, "wg")
        wu_sb = load_w(wuT, F, KD, "wu")
        wd_sb = load_w(wdT, D, MF, "wd")

        def mm1(ph, o, n, xt):
            """gate/up matmuls + silu*up -> hT tiles for one iteration."""
            g_w, u_w, wid = (wg_sb, wu_sb, F) if ph == "R" else (sg_sb, su_sb, FS)
            mf = MF if ph == "R" else MFS
            hT = []
            for m in range(mf):
                g = pgp.tile([128, NT], F32, tag="pg")
                u = pup.tile([128, NT], F32, tag="pu")
                for k in range(KD):
                    nc.tensor.matmul(g[:, :n], g_w[:, k * wid + 128 * m:
                                                  k * wid + 128 * (m + 1)],
                                     xt[:, k * NT:k * NT + n],
                                     start=(k == 0), stop=(k == KD - 1))
                for k in range(KD):
                    nc.tensor.matmul(u[:, :n], u_w[:, k * wid + 128 * m:
                                                  k * wid + 128 * (m + 1)],
                                     xt[:, k * NT:k * NT + n],
                                     start=(k == 0), stop=(k == KD - 1))
                sil = spool.tile([128, NT], F32, tag="sil")
                nc.scalar.activation(sil[:, :n], g[:, :n],
                                     mybir.ActivationFunctionType.Silu)
                h = hpool.tile([128, NT], COMPUTE_DT, tag=f"h{m}")
                nc.vector.tensor_mul(h[:, :n], sil[:, :n], u[:, :n])
                hT.append(h)
            return hT

        def mm2(ph, o, n, hT, fine=False):
            """down-projection, copy-out, one merged output DMA per tile.
            fine=True DMAs per m2-slice so the final drain isn't serialized
            behind all six copies."""
            out_dst = yeT if ph == "R" else zT
            d_w = wd_sb if ph == "R" else sd_sb
            mf = MF if ph == "R" else MFS
            yo = opool.tile([128, MD * NT], BF16, tag="yo")
            for m2 in range(MD):
                y = pyp.tile([128, NT], F32, tag="py")
                for k2 in range(mf):
                    nc.tensor.matmul(y[:, :n], d_w[:, k2 * D + 128 * m2:
                                                  k2 * D + 128 * (m2 + 1)],
                                     hT[k2][:, :n],
                                     start=(k2 == 0), stop=(k2 == mf - 1))
                ceng = nc.vector.tensor_copy if m2 % 2 == 0 else nc.scalar.copy
                ceng(yo[:, m2 * NT:m2 * NT + n], y[:, :n])
                if fine:
                    nc.sync.dma_start(out_dst[ts(m2, 128), o:o + n],
                                      yo[:, m2 * NT:m2 * NT + n])
            if not fine:
                nc.sync.dma_start(
                    out_dst[:].rearrange("(m p) j -> p m j", p=128)[:, :, o:o + n],
                    yo[:].rearrange("p (m j) -> p m j", m=MD)[:, :, :n])

        # software pipeline: emit MM1(i+1) before MM2(i) so the PE chews on
        # the next tile's gate/up while ACT/DVE finish hT(i).
        hprev = None
        for i, it in enumerate(iters):
            if i + 2 < len(iters):
                xts.append(load_x(*iters[i + 2]))
            h = mm1(*it, xts[i])
            if hprev is not None:
                mm2(*iters[i - 1], hprev)
            hprev = h
        mm2(*iters[-1], hprev, fine=True)
    nc.compile()
    return nc


def _router(xf, w_router, expert_bias):
    """Replicates the reference router. f64 for stable top-k ordering,
    f32 softmax (same formula as jax.nn.softmax) for the weights."""
    logits = xf.astype(np.float64) @ w_router.T.astype(np.float64)
    l32 = (xf @ w_router.T).astype(np.float32)
    m = l32.max(-1, keepdims=True)
    e32 = np.exp(l32 - m)
    scores = e32 / e32.sum(-1, keepdims=True)
    e64 = np.exp(logits - logits.max(-1, keepdims=True))
    sel = e64 / e64.sum(-1, keepdims=True) + expert_bias.astype(np.float64)[None, :]
    top_idx = np.argsort(-sel, axis=-1, kind="stable")[:, :TOPK]
    top_s = np.take_along_axis(scores, top_idx, axis=-1)
    top_s = top_s / (top_s.sum(-1, keepdims=True) + 1e-9)
    return top_idx, top_s


def kernel(x, w_router, expert_bias, Wg, Wu, Wd, sg, su, sd):
    x = np.asarray(x); w_router = np.asarray(w_router)
    expert_bias = np.asarray(expert_bias)
    Wg = np.asarray(Wg); Wu = np.asarray(Wu); Wd = np.asarray(Wd)
    sg = np.asarray(sg); su = np.asarray(su); sd = np.asarray(sd)
    xf = x.reshape(-1, D).astype(np.float32)

    top_idx, top_s = _router(xf, w_router, expert_bias)

    idxs, ws = [], []
    for e in range(E):
        hit = (top_idx == e)
        tok = np.nonzero(hit.any(-1))[0]
        idxs.append(tok)
        ws.append(top_s[tok][hit[tok]])
    cmax = max(len(i) for i in idxs)
    C = max(512, -(-cmax // 128) * 128)

    key = C
    if key not in _cache:
        _cache[key] = _build(C)
    nc = _cache[key]

    cast = lambda a: np.ascontiguousarray(a, dtype=np.float32).astype(NP_COMPUTE)
    in_maps = []
    for e in range(E):
        xeT = np.zeros((D, C), np.float32)
        xeT[:, :len(idxs[e])] = xf[idxs[e]].T
        th, fq = e // 4, e % 4
        in_maps.append({
            "xeT": cast(xeT),
            "wgT": cast(Wg[e].T), "wuT": cast(Wu[e].T), "wdT": cast(Wd[e].T),
            "xsT": cast(xf[th * TH:(th + 1) * TH].T),
            "sgT": cast(sg[fq * FS:(fq + 1) * FS].T),
            "suT": cast(su[fq * FS:(fq + 1) * FS].T),
            "sdT": cast(sd[:, fq * FS:(fq + 1) * FS].T),
        })

    res = run_bass_kernel_spmd(nc, in_maps, core_ids=list(range(NCORES)))

    out = np.zeros((T, D), np.float32)
    for e in range(E):
        ye = res.results[e]["yeT"].astype(np.float32).T[:len(idxs[e])]
        out[idxs[e]] += ws[e][:, None] * ye
        th = e // 4
        out[th * TH:(th + 1) * TH] += res.results[e]["zT"].astype(np.float32).T
    return out.reshape(B, S, D).astype(x.dtype)
